# revision 12
# baseline (speedup 1.0000x reference)
"""MeanStdFilter kernel for 8 Trainium2 NeuronCores.

Semantics (matches the sequential-Welford reference with M=0, S=S_in, n=0):
    S1[f] = sum_b x[b, f]            (global, over all 32768 rows)
    S2[f] = sum_b x[b, f]^2
    mean  = S1 / N
    M2    = S2 - S1^2 / N + S_in     (Welford M2 started from buffer S)
    var   = M2 / (N - 1)             (N = 32768 > 1)
    out   = (x - mean) / (sqrt(var) + 1e-5)
The input running-mean buffer M is overwritten by the first Welford step in
the reference, so it never affects the output.

Distribution: x sharded 4096 rows/core. Per core, HBM traffic is one
16.8 MB read + one 16.8 MB write (~47 us each at 360 GB/s) and the two
passes are separated by the global-stats barrier, so ~105 us is the floor.

v2 design (from the baseline-trace post-mortem):
  - Phase A (DMA-bound): load fp32 pairs of row-tiles; Scalar converts to a
    resident bf16 copy of the shard; DVE squares it (bf16 2x mode); PE
    accumulates S1/S2 with bf16 ones-matmuls (4x fewer PE cycles than fp32).
    bf16 stats noise is ~2e-5 relative on mean/std -- far under the 2e-2
    gate (baseline's fp32 collective path measured 5e-6 L2).
  - Stats exchange (was 49 us via gpsimd collective AllReduce, CC-cores are
    slow): replaced by direct remote_dma_broadcast SBUF->SBUF writes.
    Stats pack to [128, 16] (f = p*8+j); call k sends to XOR-peer (0,k)
    into gather slot k, so on receiver j slot k holds core j^k's stats --
    build-time APs, fully SPMD. Receiver waits remote_sem >= 14 (7 senders
    x 2) then reduces the 8 slots locally and finalizes in packed layout.
  - Phase C (was 79 us, DVE fp32-bound): normalize the bf16 shard in place
    (two bf16 DVE ops per chunk, 2x mode), Scalar up-converts pairs to
    fp32, DMA stores them. DVE/Scalar both fit under the 47 us store DMA.
"""

import functools
import os

import numpy as np

import concourse.bacc as bacc
import concourse.tile as tile
from concourse import mybir
from concourse.bass_utils import run_bass_kernel_spmd

NCORES = 8
B, F = 32768, 1024
ROWS = B // NCORES  # 4096 rows per core
P = 128
NT = ROWS // P  # 32 row-tiles of [128, 1024] per core
PAIR = 2  # row-tiles per load/store DMA
NPAIR = NT // PAIR  # 16
CH = 4  # row-tiles per phase-C DVE chunk
NCH = NT // CH  # 4
EPS = 1e-5
FP32 = mybir.dt.float32
BF16 = mybir.dt.bfloat16
AF = mybir.ActivationFunctionType
ALU = mybir.AluOpType


def build_kernel():
    nc = bacc.Bacc(
        "TRN2", target_bir_lowering=False, debug=False, num_devices=NCORES
    )
    x = nc.declare_dram_parameter("x", [ROWS, F], FP32, isOutput=False)
    s_in = nc.declare_dram_parameter("S", [1, F], FP32, isOutput=False)
    out = nc.declare_dram_parameter("out", [ROWS, F], FP32, isOutput=True)

    # Pair-of-tiles view: element (n, p, q, f) = x[n*256 + q*128 + p, f].
    x_pr = x[:].rearrange("(n q p) f -> n p q f", q=PAIR, p=P)
    out_pr = out[:].rearrange("(n q p) f -> n p q f", q=PAIR, p=P)
    groups = [list(range(NCORES))]

    with tile.TileContext(nc) as tc:
        with (
            tc.tile_pool(name="xload", bufs=4) as xload,
            tc.tile_pool(name="xbf", bufs=1) as xbfp,
            tc.tile_pool(name="sq", bufs=3) as sqp,
            tc.tile_pool(name="ostore", bufs=3) as ostore,
            tc.tile_pool(name="stats", bufs=1) as stats,
            tc.tile_pool(name="psum", bufs=1, space="PSUM") as psum,
            tc.tile_pool(name="dram", bufs=1, space="DRAM") as dram,
        ):
            # Warmup AllReduce: primes CC rings / absorbs core start skew
            # while the load phase runs. Result is unused. high_priority pins
            # it to the very start — the scheduler otherwise sinks it (no
            # consumers) right before the real AllReduce, wasting the overlap.
            with tc.high_priority():
                wu = stats.tile([1, 8], FP32)
                nc.vector.memset(wu, 0.0)
                wu_in = dram.tile([1, 8], FP32)
                wu_out = dram.tile([1, 8], FP32)
                sqd = stats.tile([1, 8], FP32)
                nc.scalar.activation(sqd, wu, AF.Sqrt)  # act-table preload
                nc.sync.dma_start(out=wu_in[:], in_=wu[:])
                nc.gpsimd.collective_compute(
                    "AllReduce",
                    ALU.add,
                    replica_groups=groups,
                    ins=[wu_in[:].opt()],
                    outs=[wu_out[:].opt()],
                )

            ones_bf = stats.tile([P, 1], BF16)
            nc.vector.memset(ones_bf, 1.0)

            # Resident bf16 shard (64 KB/partition).
            xbf = xbfp.tile([P, NT, F], BF16)

            # One PSUM bank per 512-wide half; accumulate across all tiles.
            ps1 = [
                psum.tile([1, 512], FP32, tag=f"ps1_{h}", name=f"ps1_{h}")
                for h in range(2)
            ]
            ps2 = [
                psum.tile([1, 512], FP32, tag=f"ps2_{h}", name=f"ps2_{h}")
                for h in range(2)
            ]

            # ---- Phase A: load, bf16-convert, square, accumulate raw sums.
            for n in range(NPAIR):
                xt = xload.tile([P, PAIR, F], FP32, tag="xt")
                nc.sync.dma_start(out=xt, in_=x_pr[n])
                xb = xbf[:, n * PAIR : (n + 1) * PAIR, :]
                nc.scalar.activation(xb, xt, AF.Copy)
                sq = sqp.tile([P, PAIR, F], BF16, tag="sq")
                nc.vector.tensor_tensor(sq[:], xb, xb, ALU.mult)
                for q in range(PAIR):
                    t = n * PAIR + q
                    for h in range(2):
                        cols = slice(h * 512, (h + 1) * 512)
                        nc.tensor.matmul(
                            ps1[h][:],
                            lhsT=ones_bf[:],
                            rhs=xbf[:, t, cols],
                            start=(t == 0),
                            stop=(t == NT - 1),
                        )
                        nc.tensor.matmul(
                            ps2[h][:],
                            lhsT=ones_bf[:],
                            rhs=sq[:, q, cols],
                            start=(t == 0),
                            stop=(t == NT - 1),
                        )

            # ---- Stats: bf16 payload halves the CC AllReduce cost; ring
            # rounding adds ~1e-3 relative on std, inside the 2e-2 budget.
            stats_sb = stats.tile([1, 2 * F], BF16)
            for h in range(2):
                nc.scalar.copy(stats_sb[:, h * 512 : (h + 1) * 512], ps1[h][:])
                nc.vector.tensor_copy(
                    stats_sb[:, F + h * 512 : F + (h + 1) * 512], ps2[h][:]
                )
            cc_in = dram.tile([1, 2 * F], BF16)
            cc_out = dram.tile([1, 2 * F], BF16)
            nc.sync.dma_start(out=cc_in[:], in_=stats_sb[:])
            nc.gpsimd.collective_compute(
                "AllReduce",
                ALU.add,
                replica_groups=groups,
                ins=[cc_in[:].opt()],
                outs=[cc_out[:].opt()],
            )

            # s_in broadcast to all partitions, prefetched off the
            # critical path (input-only dependency).
            sinb = stats.tile([P, F], FP32)
            nc.sync.dma_start(out=sinb[:], in_=s_in[:].to_broadcast([P, F]))

            # ---- Finalize redundantly in broadcast layout: one DMA hop
            # (cc_out -> every partition), then ~7 us of [128,1024] ops.
            # Replaces the packed-finalize + pack + broadcast chain whose
            # three serial DMA round-trips dominated the stats latency.
            gbc = stats.tile([P, 2 * F], BF16)
            nc.sync.dma_start(out=gbc[:], in_=cc_out[:].to_broadcast([P, 2 * F]))
            s1b = gbc[:, 0:F]
            s2b = gbc[:, F : 2 * F]
            mean_bf = stats.tile([P, F], BF16)
            rstd_bf = stats.tile([P, F], BF16)
            finw = stats.tile([P, 3 * F], FP32)
            w1, w2, w3 = (finw[:, F * i : F * (i + 1)] for i in range(3))
            nc.vector.tensor_scalar(mean_bf[:], s1b, 1.0 / B, None, ALU.mult)
            nc.vector.tensor_tensor(w1, s1b, mean_bf[:], ALU.mult)  # S1^2/N
            nc.vector.tensor_tensor(w2, s2b, w1, ALU.subtract)  # M2
            nc.vector.tensor_tensor(w2, w2, sinb[:], ALU.add)  # + S_in
            nc.scalar.activation(w3, w2, AF.Sqrt, scale=1.0 / (B - 1))  # std
            nc.scalar.activation(w1, w3, AF.Copy, bias=EPS)  # std + eps
            with nc.allow_low_precision(reason="rstd consumed in bf16"):
                nc.vector.reciprocal(rstd_bf[:], w1)

            # ---- Phase C: normalize bf16 shard in place, upconvert, store.
            for c in range(NCH):
                xc = xbf[:, c * CH : (c + 1) * CH, :]
                mb = mean_bf[:, None, :].to_broadcast([P, CH, F])
                rb = rstd_bf[:, None, :].to_broadcast([P, CH, F])
                nc.vector.tensor_tensor(xc, xc, mb, ALU.subtract)
                nc.vector.tensor_tensor(xc, xc, rb, ALU.mult)
                for m in range(CH // PAIR):
                    n = c * (CH // PAIR) + m
                    ot = ostore.tile([P, PAIR, F], FP32, tag="ot")
                    nc.scalar.activation(
                        ot, xbf[:, n * PAIR : (n + 1) * PAIR, :], AF.Copy
                    )
                    nc.sync.dma_start(out=out_pr[n], in_=ot)

    nc.finalize()
    return nc


@functools.cache
def _get_nc():
    return build_kernel()


def kernel(x, M, S, _trace=False, _trace_kwargs=None):
    del M  # overwritten by the first Welford step in the reference
    x = np.ascontiguousarray(x, dtype=np.float32)
    S = np.ascontiguousarray(S, dtype=np.float32).reshape(1, F)
    nc = _get_nc()
    in_maps = [
        {"x": x[i * ROWS : (i + 1) * ROWS], "S": S} for i in range(NCORES)
    ]
    res = run_bass_kernel_spmd(
        nc,
        in_maps,
        core_ids=list(range(NCORES)),
        trace=_trace,
        **(_trace_kwargs or {}),
    )
    out = np.concatenate([res.results[i]["out"] for i in range(NCORES)], axis=0)
    if _trace:
        return out, res
    return out


# revision 13
# speedup vs baseline: 1.6116x; 1.6116x over previous
"""MeanStdFilter kernel for 8 Trainium2 NeuronCores.

Semantics (matches the sequential-Welford reference with M=0, S=S_in, n=0):
    S1[f] = sum_b x[b, f]            (global, over all 32768 rows)
    S2[f] = sum_b x[b, f]^2
    mean  = S1 / N
    M2    = S2 - S1^2 / N + S_in     (Welford M2 started from buffer S)
    var   = M2 / (N - 1)             (N = 32768 > 1)
    out   = (x - mean) / (sqrt(var) + 1e-5)
The input running-mean buffer M is overwritten by the first Welford step in
the reference, so it never affects the output.

Distribution: x sharded 4096 rows/core. Per-core HBM traffic is one 16.8 MB
read + one 16.8 MB write (~53 us each at the ~317 GB/s/core practical rate),
serialized by the global-stats barrier; ~120 us is the hard floor.

Design (from baseline-trace post-mortems; baseline was 200-209 us):
  - Phase A (load-DMA-bound, ~53 us): load fp32 pairs of row-tiles; Scalar
    down-converts into a resident bf16 shard copy; DVE squares it (bf16 2x
    mode, 1.2 us/pair); PE accumulates S1/S2 with bf16 ones-matmuls (1
    cyc/row vs fp32's 4). Engines all sit under the DMA pace.
  - Stats AllReduce in bf16 (4 KB payload): CC-core busy time drops to
    ~12 us from fp32's ~19-27. Ring rounding adds ~1e-3 rel on std --
    inside the 2e-2 budget (measured L2 3.3e-3 total).
  - Finalize (~8 us): cc_out is DMA-broadcast once to all 128 partitions
    and mean/rstd are computed redundantly at [128, 1024(x)] -- one DMA hop
    on the critical path instead of the packed-finalize's three round-trips
    (which measured ~16 us). A dummy Sqrt at kernel start pins the single
    act table load off the critical path.
  - Phase C (store-DMA-bound, ~52 us): normalize the bf16 shard in place
    (two bf16 2x DVE ops per 4-tile chunk), Scalar up-converts pairs to
    fp32 work tiles, DMA stores them; converts/stores pipeline at the
    2.95 us/pair store pace.
  - The warmup AllReduce is pinned to the start via tc.high_priority() --
    left to the scheduler it sinks to just before the real AllReduce (no
    consumers) and the first-collective init lands on the critical path
    (measured 396 us total that way).

Remaining variance (174-296 us observed): per-core launch/NEFF-init skew
surfaces at the AllReduce rendezvous and compounds via cross-core HBM
contention. It is environment noise, not kernel-controlled.
"""

import functools

import numpy as np

import concourse.bacc as bacc
import concourse.tile as tile
from concourse import mybir
from concourse.bass_utils import run_bass_kernel_spmd

NCORES = 8
B, F = 32768, 1024
ROWS = B // NCORES  # 4096 rows per core
P = 128
NT = ROWS // P  # 32 row-tiles of [128, 1024] per core
PAIR = 2  # row-tiles per load/store DMA
NPAIR = NT // PAIR  # 16
CH = 4  # row-tiles per phase-C DVE chunk
NCH = NT // CH  # 4
EPS = 1e-5
FP32 = mybir.dt.float32
BF16 = mybir.dt.bfloat16
AF = mybir.ActivationFunctionType
ALU = mybir.AluOpType


def build_kernel():
    nc = bacc.Bacc(
        "TRN2", target_bir_lowering=False, debug=False, num_devices=NCORES
    )
    x = nc.declare_dram_parameter("x", [ROWS, F], FP32, isOutput=False)
    s_in = nc.declare_dram_parameter("S", [1, F], FP32, isOutput=False)
    out = nc.declare_dram_parameter("out", [ROWS, F], FP32, isOutput=True)

    # Pair-of-tiles view: element (n, p, q, f) = x[n*256 + q*128 + p, f].
    x_pr = x[:].rearrange("(n q p) f -> n p q f", q=PAIR, p=P)
    out_pr = out[:].rearrange("(n q p) f -> n p q f", q=PAIR, p=P)
    groups = [list(range(NCORES))]

    with tile.TileContext(nc) as tc:
        with (
            tc.tile_pool(name="xload", bufs=4) as xload,
            tc.tile_pool(name="xbf", bufs=1) as xbfp,
            tc.tile_pool(name="sq", bufs=3) as sqp,
            tc.tile_pool(name="ostore", bufs=3) as ostore,
            tc.tile_pool(name="stats", bufs=1) as stats,
            tc.tile_pool(name="psum", bufs=1, space="PSUM") as psum,
            tc.tile_pool(name="dram", bufs=1, space="DRAM") as dram,
        ):
            # Warmup AllReduce: primes CC rings / absorbs core start skew
            # while the load phase runs. Result is unused. high_priority pins
            # it to the very start — the scheduler otherwise sinks it (no
            # consumers) right before the real AllReduce, wasting the overlap.
            with tc.high_priority():
                wu = stats.tile([1, 8], FP32)
                nc.vector.memset(wu, 0.0)
                wu_in = dram.tile([1, 8], FP32)
                wu_out = dram.tile([1, 8], FP32)
                sqd = stats.tile([1, 8], FP32)
                nc.scalar.activation(sqd, wu, AF.Sqrt)  # act-table preload
                nc.sync.dma_start(out=wu_in[:], in_=wu[:])
                nc.gpsimd.collective_compute(
                    "AllReduce",
                    ALU.add,
                    replica_groups=groups,
                    ins=[wu_in[:].opt()],
                    outs=[wu_out[:].opt()],
                )

            ones_bf = stats.tile([P, 1], BF16)
            nc.vector.memset(ones_bf, 1.0)

            # Resident bf16 shard (64 KB/partition).
            xbf = xbfp.tile([P, NT, F], BF16)

            # One PSUM bank per 512-wide half; accumulate across all tiles.
            ps1 = [
                psum.tile([1, 512], FP32, tag=f"ps1_{h}", name=f"ps1_{h}")
                for h in range(2)
            ]
            ps2 = [
                psum.tile([1, 512], FP32, tag=f"ps2_{h}", name=f"ps2_{h}")
                for h in range(2)
            ]

            # ---- Phase A: load, bf16-convert, square, accumulate raw sums.
            for n in range(NPAIR):
                xt = xload.tile([P, PAIR, F], FP32, tag="xt")
                nc.sync.dma_start(out=xt, in_=x_pr[n])
                xb = xbf[:, n * PAIR : (n + 1) * PAIR, :]
                nc.scalar.activation(xb, xt, AF.Copy)
                sq = sqp.tile([P, PAIR, F], BF16, tag="sq")
                nc.vector.tensor_tensor(sq[:], xb, xb, ALU.mult)
                for q in range(PAIR):
                    t = n * PAIR + q
                    for h in range(2):
                        cols = slice(h * 512, (h + 1) * 512)
                        nc.tensor.matmul(
                            ps1[h][:],
                            lhsT=ones_bf[:],
                            rhs=xbf[:, t, cols],
                            start=(t == 0),
                            stop=(t == NT - 1),
                        )
                        nc.tensor.matmul(
                            ps2[h][:],
                            lhsT=ones_bf[:],
                            rhs=sq[:, q, cols],
                            start=(t == 0),
                            stop=(t == NT - 1),
                        )

            # ---- Stats: bf16 payload halves the CC AllReduce cost; ring
            # rounding adds ~1e-3 relative on std, inside the 2e-2 budget.
            stats_sb = stats.tile([1, 2 * F], BF16)
            for h in range(2):
                nc.scalar.copy(stats_sb[:, h * 512 : (h + 1) * 512], ps1[h][:])
                nc.vector.tensor_copy(
                    stats_sb[:, F + h * 512 : F + (h + 1) * 512], ps2[h][:]
                )
            cc_in = dram.tile([1, 2 * F], BF16)
            cc_out = dram.tile([1, 2 * F], BF16)
            nc.sync.dma_start(out=cc_in[:], in_=stats_sb[:])
            nc.gpsimd.collective_compute(
                "AllReduce",
                ALU.add,
                replica_groups=groups,
                ins=[cc_in[:].opt()],
                outs=[cc_out[:].opt()],
            )

            # s_in broadcast to all partitions, prefetched off the
            # critical path (input-only dependency).
            sinb = stats.tile([P, F], FP32)
            nc.sync.dma_start(out=sinb[:], in_=s_in[:].to_broadcast([P, F]))

            # ---- Finalize redundantly in broadcast layout: one DMA hop
            # (cc_out -> every partition), then ~7 us of [128,1024] ops.
            # Replaces the packed-finalize + pack + broadcast chain whose
            # three serial DMA round-trips dominated the stats latency.
            gbc = stats.tile([P, 2 * F], BF16)
            nc.sync.dma_start(out=gbc[:], in_=cc_out[:].to_broadcast([P, 2 * F]))
            s1b = gbc[:, 0:F]
            s2b = gbc[:, F : 2 * F]
            mean_bf = stats.tile([P, F], BF16)
            rstd_bf = stats.tile([P, F], BF16)
            finw = stats.tile([P, 3 * F], FP32)
            w1, w2, w3 = (finw[:, F * i : F * (i + 1)] for i in range(3))
            nc.vector.tensor_scalar(mean_bf[:], s1b, 1.0 / B, None, ALU.mult)
            nc.vector.tensor_tensor(w1, s1b, mean_bf[:], ALU.mult)  # S1^2/N
            nc.vector.tensor_tensor(w2, s2b, w1, ALU.subtract)  # M2
            nc.vector.tensor_tensor(w2, w2, sinb[:], ALU.add)  # + S_in
            nc.scalar.activation(w3, w2, AF.Sqrt, scale=1.0 / (B - 1))  # std
            nc.scalar.activation(w1, w3, AF.Copy, bias=EPS)  # std + eps
            with nc.allow_low_precision(reason="rstd consumed in bf16"):
                nc.vector.reciprocal(rstd_bf[:], w1)

            # ---- Phase C: normalize bf16 shard in place, upconvert, store.
            for c in range(NCH):
                xc = xbf[:, c * CH : (c + 1) * CH, :]
                mb = mean_bf[:, None, :].to_broadcast([P, CH, F])
                rb = rstd_bf[:, None, :].to_broadcast([P, CH, F])
                nc.vector.tensor_tensor(xc, xc, mb, ALU.subtract)
                nc.vector.tensor_tensor(xc, xc, rb, ALU.mult)
                for m in range(CH // PAIR):
                    n = c * (CH // PAIR) + m
                    ot = ostore.tile([P, PAIR, F], FP32, tag="ot")
                    nc.scalar.activation(
                        ot, xbf[:, n * PAIR : (n + 1) * PAIR, :], AF.Copy
                    )
                    nc.sync.dma_start(out=out_pr[n], in_=ot)

    nc.finalize()
    return nc


@functools.cache
def _get_nc():
    return build_kernel()


def kernel(x, M, S, _trace=False, _trace_kwargs=None):
    del M  # overwritten by the first Welford step in the reference
    x = np.ascontiguousarray(x, dtype=np.float32)
    S = np.ascontiguousarray(S, dtype=np.float32).reshape(1, F)
    nc = _get_nc()
    in_maps = [
        {"x": x[i * ROWS : (i + 1) * ROWS], "S": S} for i in range(NCORES)
    ]
    res = run_bass_kernel_spmd(
        nc,
        in_maps,
        core_ids=list(range(NCORES)),
        trace=_trace,
        **(_trace_kwargs or {}),
    )
    out = np.concatenate([res.results[i]["out"] for i in range(NCORES)], axis=0)
    if _trace:
        return out, res
    return out


# revision 14
# speedup vs baseline: 1.6770x; 1.0406x over previous
"""MeanStdFilter kernel for 8 Trainium2 NeuronCores.

Semantics (matches the sequential-Welford reference with M=0, S=S_in, n=0):
    S1[f] = sum_b x[b, f]            (global, over all 32768 rows)
    S2[f] = sum_b x[b, f]^2
    mean  = S1 / N
    M2    = S2 - S1^2 / N + S_in     (Welford M2 started from buffer S)
    var   = M2 / (N - 1)             (N = 32768 > 1)
    out   = (x - mean) / (sqrt(var) + 1e-5)
The input running-mean buffer M is overwritten by the first Welford step in
the reference, so it never affects the output.

Distribution: x sharded 4096 rows/core. Per-core HBM traffic is one 16.8 MB
read + one 16.8 MB write (~53 us each at the ~317 GB/s/core practical rate),
serialized by the global-stats barrier; ~120 us is the hard floor.

Design (from baseline-trace post-mortems; baseline was 200-209 us):
  - Phase A (load-DMA-bound, ~53 us): load fp32 pairs of row-tiles; Scalar
    down-converts into a resident bf16 shard copy; DVE squares it (bf16 2x
    mode, 1.2 us/pair); PE accumulates S1/S2 with bf16 ones-matmuls (1
    cyc/row vs fp32's 4). Engines all sit under the DMA pace.
  - Stats AllReduce in bf16 (4 KB payload): CC-core busy time drops to
    ~12 us from fp32's ~19-27. Ring rounding adds ~1e-3 rel on std --
    inside the 2e-2 budget (measured L2 3.3e-3 total).
  - Finalize (~8 us): cc_out is DMA-broadcast once to all 128 partitions
    and mean/rstd are computed redundantly at [128, 1024(x)] -- one DMA hop
    on the critical path instead of the packed-finalize's three round-trips
    (which measured ~16 us). A dummy Sqrt at kernel start pins the single
    act table load off the critical path.
  - Phase C (store-DMA-bound, ~52 us): normalize the bf16 shard in place
    (two bf16 2x DVE ops per 4-tile chunk), Scalar up-converts pairs to
    fp32 work tiles, DMA stores them; converts/stores pipeline at the
    2.95 us/pair store pace.
  - The warmup AllReduce is pinned to the start via tc.high_priority() --
    left to the scheduler it sinks to just before the real AllReduce (no
    consumers) and the first-collective init lands on the critical path
    (measured 396 us total that way).

Remaining variance (174-296 us observed): per-core launch/NEFF-init skew
surfaces at the AllReduce rendezvous and compounds via cross-core HBM
contention. It is environment noise, not kernel-controlled.
"""

import functools

import numpy as np

import concourse.bacc as bacc
import concourse.tile as tile
from concourse import mybir
from concourse.bass_utils import run_bass_kernel_spmd

NCORES = 8
B, F = 32768, 1024
ROWS = B // NCORES  # 4096 rows per core
P = 128
NT = ROWS // P  # 32 row-tiles of [128, 1024] per core
PAIR = 2  # row-tiles per load/store DMA
NPAIR = NT // PAIR  # 16
CH = 4  # row-tiles per phase-C DVE chunk
NCH = NT // CH  # 4
EPS = 1e-5
FP32 = mybir.dt.float32
BF16 = mybir.dt.bfloat16
AF = mybir.ActivationFunctionType
ALU = mybir.AluOpType


def build_kernel():
    nc = bacc.Bacc(
        "TRN2", target_bir_lowering=False, debug=False, num_devices=NCORES
    )
    x = nc.declare_dram_parameter("x", [ROWS, F], FP32, isOutput=False)
    s_in = nc.declare_dram_parameter("S", [1, F], FP32, isOutput=False)
    out = nc.declare_dram_parameter("out", [ROWS, F], FP32, isOutput=True)

    # Tile views: x_t[t] is row-tile t ([128, F]); x_pr[n] is a pair of
    # tiles ([128, 2, F]).  Element (n, p, q, f) = x[n*256 + q*128 + p, f].
    x_t = x[:].rearrange("(n p) f -> n p f", p=P)
    x_pr = x[:].rearrange("(n q p) f -> n p q f", q=PAIR, p=P)
    out_pr = out[:].rearrange("(n q p) f -> n p q f", q=PAIR, p=P)
    groups = [list(range(NCORES))]

    with tile.TileContext(nc) as tc:
        with (
            tc.tile_pool(name="xload", bufs=4) as xload,
            tc.tile_pool(name="xbf", bufs=1) as xbfp,
            tc.tile_pool(name="sq", bufs=3) as sqp,
            tc.tile_pool(name="ostore", bufs=3) as ostore,
            tc.tile_pool(name="stats", bufs=1) as stats,
            tc.tile_pool(name="psum", bufs=1, space="PSUM") as psum,
            tc.tile_pool(name="dram", bufs=1, space="DRAM") as dram,
        ):
            # Warmup AllReduce: primes CC rings / absorbs core start skew
            # while the load phase runs. Result is unused. high_priority pins
            # it to the very start — the scheduler otherwise sinks it (no
            # consumers) right before the real AllReduce, wasting the overlap.
            with tc.high_priority():
                wu = stats.tile([1, 8], FP32)
                nc.vector.memset(wu, 0.0)
                wu_in = dram.tile([1, 8], FP32)
                wu_out = dram.tile([1, 8], FP32)
                sqd = stats.tile([1, 8], FP32)
                nc.scalar.activation(sqd, wu, AF.Sqrt)  # act-table preload
                nc.sync.dma_start(out=wu_in[:], in_=wu[:])
                nc.gpsimd.collective_compute(
                    "AllReduce",
                    ALU.add,
                    replica_groups=groups,
                    ins=[wu_in[:].opt()],
                    outs=[wu_out[:].opt()],
                )

            ones_bf = stats.tile([P, 1], BF16)
            nc.vector.memset(ones_bf, 1.0)

            # Resident bf16 shard (64 KB/partition).
            xbf = xbfp.tile([P, NT, F], BF16)

            # One PSUM bank per 512-wide half; accumulate across all tiles.
            ps1 = [
                psum.tile([1, 512], FP32, tag=f"ps1_{h}", name=f"ps1_{h}")
                for h in range(2)
            ]
            ps2 = [
                psum.tile([1, 512], FP32, tag=f"ps2_{h}", name=f"ps2_{h}")
                for h in range(2)
            ]

            # ---- Phase A: load, bf16-convert, square, accumulate raw sums.
            # Single-tile groups at both ends shorten pipeline fill (first
            # convert starts a tile-load earlier) and drain (shorter tail
            # after the last load).
            groups_a = [1, 1] + [PAIR] * ((NT - 4) // PAIR) + [1, 1]
            t0 = 0
            for g in groups_a:
                xt = xload.tile([P, g, F], FP32, tag=f"xt{g}")
                if g == 1:
                    nc.sync.dma_start(out=xt[:, 0, :], in_=x_t[t0])
                else:
                    nc.sync.dma_start(out=xt, in_=x_pr[t0 // PAIR])
                xb = xbf[:, t0 : t0 + g, :]
                nc.scalar.activation(xb, xt, AF.Copy)
                sq = sqp.tile([P, g, F], BF16, tag=f"sq{g}")
                nc.vector.tensor_tensor(sq[:], xb, xb, ALU.mult)
                for q in range(g):
                    t = t0 + q
                    for h in range(2):
                        cols = slice(h * 512, (h + 1) * 512)
                        nc.tensor.matmul(
                            ps1[h][:],
                            lhsT=ones_bf[:],
                            rhs=xbf[:, t, cols],
                            start=(t == 0),
                            stop=(t == NT - 1),
                        )
                        nc.tensor.matmul(
                            ps2[h][:],
                            lhsT=ones_bf[:],
                            rhs=sq[:, q, cols],
                            start=(t == 0),
                            stop=(t == NT - 1),
                        )
                t0 += g

            # ---- Stats: bf16 payload halves the CC AllReduce cost; ring
            # rounding adds ~1e-3 relative on std, inside the 2e-2 budget.
            stats_sb = stats.tile([1, 2 * F], BF16)
            for h in range(2):
                nc.scalar.copy(stats_sb[:, h * 512 : (h + 1) * 512], ps1[h][:])
                nc.vector.tensor_copy(
                    stats_sb[:, F + h * 512 : F + (h + 1) * 512], ps2[h][:]
                )
            cc_in = dram.tile([1, 2 * F], BF16)
            cc_out = dram.tile([1, 2 * F], BF16)
            nc.sync.dma_start(out=cc_in[:], in_=stats_sb[:])
            nc.gpsimd.collective_compute(
                "AllReduce",
                ALU.add,
                replica_groups=groups,
                ins=[cc_in[:].opt()],
                outs=[cc_out[:].opt()],
            )

            # s_in broadcast to all partitions, prefetched and converted
            # to bf16 off the critical path (input-only dependency).
            sinb = stats.tile([P, F], FP32)
            nc.sync.dma_start(out=sinb[:], in_=s_in[:].to_broadcast([P, F]))
            sinb_bf = stats.tile([P, F], BF16)
            nc.vector.tensor_copy(sinb_bf[:], sinb[:])

            # ---- Finalize redundantly in broadcast layout: one DMA hop
            # (cc_out -> every partition), then ~7 us of [128,1024] ops.
            # Replaces the packed-finalize + pack + broadcast chain whose
            # three serial DMA round-trips dominated the stats latency.
            gbc = stats.tile([P, 2 * F], BF16)
            nc.sync.dma_start(out=gbc[:], in_=cc_out[:].to_broadcast([P, 2 * F]))
            s1b = gbc[:, 0:F]
            s2b = gbc[:, F : 2 * F]
            mean_bf = stats.tile([P, F], BF16)
            rstd_bf = stats.tile([P, F], BF16)
            wbf = stats.tile([P, 2 * F], BF16)
            w1, w2 = wbf[:, 0:F], wbf[:, F : 2 * F]
            finw = stats.tile([P, 2 * F], FP32)
            w3, w4 = finw[:, 0:F], finw[:, F : 2 * F]
            # DVE chain all-bf16 (2x mode); the AR payload is already bf16
            # so the extra rounding on M2 (~0.2%) is below existing noise.
            nc.vector.tensor_scalar(mean_bf[:], s1b, 1.0 / B, None, ALU.mult)
            nc.vector.tensor_tensor(w1, s1b, mean_bf[:], ALU.mult)  # S1^2/N
            nc.vector.tensor_tensor(w2, s2b, w1, ALU.subtract)  # M2
            nc.vector.tensor_tensor(w2, w2, sinb_bf[:], ALU.add)  # + S_in
            nc.scalar.activation(w3, w2, AF.Sqrt, scale=1.0 / (B - 1))  # std
            nc.scalar.activation(w4, w3, AF.Copy, bias=EPS)  # std + eps
            with nc.allow_low_precision(reason="rstd consumed in bf16"):
                nc.vector.reciprocal(rstd_bf[:], w4)

            # ---- Phase C: normalize bf16 shard in place, upconvert, store.
            # Graded chunks: small first chunks get the store pipeline
            # going a few us sooner after the stats land.
            chunks_c = [2, 2, 4, 4, 4, 8, 8]
            assert sum(chunks_c) == NT
            s0 = 0
            for c in chunks_c:
                xc = xbf[:, s0 : s0 + c, :]
                mb = mean_bf[:, None, :].to_broadcast([P, c, F])
                rb = rstd_bf[:, None, :].to_broadcast([P, c, F])
                nc.vector.tensor_tensor(xc, xc, mb, ALU.subtract)
                nc.vector.tensor_tensor(xc, xc, rb, ALU.mult)
                for m in range(c // PAIR):
                    n = s0 // PAIR + m
                    ot = ostore.tile([P, PAIR, F], FP32, tag="ot")
                    nc.scalar.activation(
                        ot, xbf[:, n * PAIR : (n + 1) * PAIR, :], AF.Copy
                    )
                    nc.sync.dma_start(out=out_pr[n], in_=ot)
                s0 += c

    nc.finalize()
    return nc


@functools.cache
def _get_nc():
    return build_kernel()


def kernel(x, M, S, _trace=False, _trace_kwargs=None):
    del M  # overwritten by the first Welford step in the reference
    x = np.ascontiguousarray(x, dtype=np.float32)
    S = np.ascontiguousarray(S, dtype=np.float32).reshape(1, F)
    nc = _get_nc()
    in_maps = [
        {"x": x[i * ROWS : (i + 1) * ROWS], "S": S} for i in range(NCORES)
    ]
    res = run_bass_kernel_spmd(
        nc,
        in_maps,
        core_ids=list(range(NCORES)),
        trace=_trace,
        **(_trace_kwargs or {}),
    )
    out = np.concatenate([res.results[i]["out"] for i in range(NCORES)], axis=0)
    if _trace:
        return out, res
    return out
